# revision 1
# baseline (speedup 1.0000x reference)
"""Trainium2 Bass kernel for nn_CBSA_45389214384209 (sparse_attention).

Reference computation (per batch element b of 8):
  x_seq = x[b].T                      # [4096, 256]   (x[b] is [256, 4096])
  proj  = x_seq @ W_proj              # [4096, 512]
  rep   = avgpool8x8(proj)            # [64, 512]
  per head h (8 heads, dh=64):
    S    = rep_h @ proj_h.T * scale   # [64, 4096]
    P    = softmax(S)                 # [64, 4096]
    rd   = P @ proj_h                 # [64, 64]
    rep2 = rep_h + step_rep[h] * rd
    P2   = softmax(rep2 @ rep2.T * scale)
    xd2  = step_x[h] * (P2 @ rep2)    # [64, 64]
    xdT  = xd2.T @ P                  # [64, 4096]  (back-projection)
  out[b] = W_out.T @ concat_h(xdT) + b_out[:, None]   # [256, 4096]

Kernel structure (heads packed in pairs into 128-wide tiles throughout):
  * Pooling commutes with the projection: rep^T = Wp^T pool8x8(x) / 64, so
    the pooled queries exist as soon as the x DMAs land (~16us), long
    before any projection math.
  * Scores come straight from x via fused weights: S_p = (Wp repT_bd_p)^T x
    with repT_bd the SCALE-scaled block-diagonal pooled queries -- the
    transposed projection (projT) is never materialized, which removes
    ~14us of PE matmuls and ~21us of PSUM->SBUF export.  exp runs as one
    unbroken ACT chain threaded through the projection loop.
  * P^T (for rep_delta) comes from piecewise DMA-xbar transposes issued
    right behind the exp chain; proj in token-partition layout (rep_delta
    lhsT) is computed as a second projection pass from the f32r-staged x.
  * Stage 2 uses exp(S2) symmetry: S2^T is computed with swapped matmul
    operands and exp-ed directly into the [64,128] layout (no P2 transpose,
    no P2 normalization -- 1/Z2 folds into the V row scale), and xd2^T is
    produced directly by swapping the xd2 matmul operands.
  * Back-projection + output projection are fused algebraically:
    out = sum_h (Wo_h^T xd2_h^T) @ P_h + b_out, via per-pair V^T =
    xd2_bd^T @ Wo_pair -- the [64,4096] back-projection never materializes
    and the output stage is a single K=512 accumulation over 4 pairs.
  * Engine budget: PE ~54us; DVE carries xpool + PSUM exports; ACT carries
    the exp chain; Pool (GPSIMD, SBUF-only) carries the f32r casts.

Sharding: pure data parallel - one batch element per NeuronCore (8 cores).
"""

import os
import sys

import numpy as np

for _p in ("/opt/trn_rl_repo", os.path.expanduser("~/.axon_site/_ro/trn_rl_repo")):
    if os.path.isdir(_p) and _p not in sys.path:
        sys.path.insert(0, _p)

import concourse.bass as bass
import concourse.tile as tile
from concourse import bacc, mybir
from concourse.bass import ds, ts
from concourse.masks import make_identity

F32 = mybir.dt.float32
F32R = mybir.dt.float32r
BF16 = mybir.dt.bfloat16
AX = mybir.AxisListType
ALU = mybir.AluOpType
ACTF = mybir.ActivationFunctionType

B = 8
C = 256          # model dim
T = 4096         # tokens (64x64 grid)
INNER = 512
HEADS = 8
DH = 64
NB = 64          # pooled tokens (8x8 grid)
SCALE = DH ** -0.5
NPAIR = 4        # head pairs
NCHUNK = 8       # 512-wide token chunks
NTT = 32         # 128-wide token tiles

CFG = {"p_mode": "bf16"}


def build_module(cfg=CFG):
    nc = bacc.Bacc("TRN2", debug=False)

    x = nc.dram_tensor("x", [C, T], F32, kind="ExternalInput").ap()
    wp = nc.dram_tensor("w_proj", [C, INNER], F32, kind="ExternalInput").ap()
    wo = nc.dram_tensor("w_out", [INNER, C], F32, kind="ExternalInput").ap()
    bo = nc.dram_tensor("b_out", [C], F32, kind="ExternalInput").ap()
    srep = nc.dram_tensor("s_rep", [HEADS], F32, kind="ExternalInput").ap()
    sx = nc.dram_tensor("s_x", [HEADS], F32, kind="ExternalInput").ap()
    out = nc.dram_tensor("out", [C, T], F32, kind="ExternalOutput").ap()

    with tile.TileContext(nc) as tc:
        _body(tc, cfg, x, wp, wo, bo, srep, sx, out)
    nc.compile()
    return nc


def _body(tc, cfg, x, wp, wo, bo, srep, sx, out):
    nc = tc.nc

    x_r = x.rearrange("(o p) t -> p o t", p=128)      # [128, 2, 4096]
    out_r = out.rearrange("(o p) t -> p o t", p=128)  # [128, 2, 4096]
    wp_v = wp.rearrange("(o p) i -> p o i", p=128)    # [128, 2, 512]

    # ---- pools (SBUF pools stack-nested: alloc order == reverse release) --
    # xp (stack top) is released after the projection pass; the out staging
    # pool is allocated afterwards and reuses its space.
    consts = tc.alloc_tile_pool(name="consts", bufs=1)
    stats = tc.alloc_tile_pool(name="stats", bufs=1)
    vp = tc.alloc_tile_pool(name="vp", bufs=1)           # V^T per pair
    pp = tc.alloc_tile_pool(name="pp", bufs=1)           # P (attn) tiles
    b3 = tc.alloc_tile_pool(name="b3", bufs=2)           # stage-2 temps
    ptp = tc.alloc_tile_pool(name="ptp", bufs=1)         # P^T (rotating)
    pnp = tc.alloc_tile_pool(name="pnp", bufs=1)         # proj (t-partition)
    pTp = tc.alloc_tile_pool(name="pTp", bufs=1)         # projT (bf16)
    xp = tc.alloc_tile_pool(name="xp", bufs=1)           # x staging (f32r)

    psum = tc.alloc_tile_pool(name="psum", bufs=1, space="PSUM")
    mm_bufs = 3

    # ---- weights first: W_proj o=0 half gates the first matmul -----------
    # Interleave the chunk-0 halves so PE can start ~2us in.
    wp_sb = consts.tile([128, 2, INNER], F32, name="wp_sb")
    wp_r = consts.tile([128, 2, INNER], F32R, name="wp_r")
    xc_tiles = [
        xp.tile([128, 2, 512], F32, name="xc", tag="xc", bufs=4)
        for _ in range(NCHUNK)
    ]
    nc.sync.dma_start(wp_sb[:, 0, :], wp_v[:, 0, :])
    nc.sync.dma_start(xc_tiles[0][:, 0, :], x_r[:, 0, ts(0, 512)])
    nc.vector.tensor_copy(wp_r[:, 0, :], wp_sb[:, 0, :])
    nc.sync.dma_start(wp_sb[:, 1, :], wp_v[:, 1, :])
    nc.sync.dma_start(xc_tiles[0][:, 1, :], x_r[:, 1, ts(0, 512)])
    nc.vector.tensor_copy(wp_r[:, 1, :], wp_sb[:, 1, :])
    nc.sync.dma_start(xc_tiles[1], x_r[:, :, ts(1, 512)])

    ident_bf = consts.tile([128, 128], BF16, name="ident_bf")
    make_identity(nc, ident_bf)
    ident_f = consts.tile([128, 128], F32, name="ident_f")
    make_identity(nc, ident_f)

    # Wp^T (f32r), for the fused score weights Ws = Wp @ repT_bd
    wpT_sb = consts.tile([128, 4, C], F32R, name="wpT_sb")
    for k in range(4):
        for o in range(2):
            wt_ps = psum.tile([128, 128], F32, name="wt_ps", tag="sm", bufs=2)
            nc.tensor.transpose(wt_ps, wp_sb[:, o, ts(k, 128)], ident_f)
            nc.vector.tensor_copy(wpT_sb[:, k, ds(128 * o, 128)], wt_ps)
    ws_sb = consts.tile([128, 2, NPAIR, 128], F32R, name="ws_sb")

    # ---- pass over x: projT = Wp^T x AND proj = x^T Wp, interleaved ------
    # Both layouts of the projection are computed per 512-token chunk (the
    # doubled PE stream hides the cross-engine copies).  The pooled tokens
    # come from pooling x itself (pooling commutes with W_proj): xpool is
    # complete as soon as the x DMAs land (~1/3 into this pass), so repT,
    # the scores and the exp chain can all overlap the projection pass.
    xpool_raw = consts.tile([128, 2, NB], F32, name="xpool_raw")
    xpool_r = consts.tile([128, 2, NB], F32R, name="xpool_r")
    repT = consts.tile([128, NPAIR, NB], F32, name="repT")
    repT_bd = consts.tile([128, NPAIR, 128], F32, name="repT_bd")
    repT_bd_r = consts.tile([128, NPAIR, 128], F32R, name="repT_bd_r")
    proj_bf = pnp.tile([128, NTT, INNER], BF16, name="proj_bf")
    x_sb = xp.tile([128, 2, T], F32R, name="x_sb")
    p_tiles = [
        pp.tile([128, T], BF16, name=f"p{p}", tag=f"p{p}") for p in range(NPAIR)
    ]
    zpart_tiles = [
        stats.tile([128, NCHUNK], F32, name="zpart", tag=f"zpart{p}")
        for p in range(NPAIR)
    ]
    pt_tiles = [
        ptp.tile([128, NTT, 128], BF16, name=f"pt{p}", tag="pt", bufs=4)
        for p in range(NPAIR)
    ]

    # prologue: x load + 8x8 block pooling of x, paced by DMA (~16us) -- so
    # the pooled queries exist ~1/3 into the projection pass.  Casts for
    # chunks 0-3 run on ACT here (exp needs ACT free from ~18us on); chunks
    # 4-7 are cast on Pool inside the projection loop.
    for j in range(NCHUNK):
        xc = xc_tiles[j]
        if j >= 2:
            nc.sync.dma_start(xc, x_r[:, :, ts(j, 512)])
        if j == 0:
            # per-half casts on separate engines: o=0 unblocks matmul 0 early
            nc.vector.tensor_copy(x_sb[:, 0, ts(0, 512)], xc[:, 0, :])
            nc.scalar.copy(x_sb[:, 1, ts(0, 512)], xc[:, 1, :])
        else:
            nc.gpsimd.tensor_copy(x_sb[:, :, ts(j, 512)], xc)
        # block sums of x: chunk j covers h rows 8j..8j+8 (one block row);
        # local t = hi*64 + wb*8 + wi -> reduce (hi, wi) per block col wb
        nc.vector.reduce_sum(
            xpool_raw[:, :, ts(j, 8)],
            xc.rearrange("p o (hi wb wi) -> p o wb hi wi", hi=8, wb=8, wi=8),
            axis=AX.XY,
        )

    # remaining constants (triggered after the x chunks on the DMA queue)
    wo_sb = consts.tile([128, 4, C], F32, name="wo_sb")
    nc.sync.dma_start(wo_sb, wo.rearrange("(g p) c -> p g c", p=128))
    wo_bf = consts.tile([128, 4, C], BF16, name="wo_bf")
    nc.gpsimd.tensor_copy(wo_bf, wo_sb)
    bo_ld = consts.tile([128, 2], F32, name="bo_ld")
    nc.sync.dma_start(bo_ld, bo.rearrange("(o p) -> p o", p=128))
    bo_sb = consts.tile([128, 2], F32, name="bo_sb")
    nc.gpsimd.tensor_copy(bo_sb, bo_ld)
    # step_rep / step_x broadcast per pair: column p holds step[2p] on
    # partitions 0-63 and step[2p+1] on partitions 64-127.
    ones_row = consts.tile([1, 128], F32, name="ones_row")
    nc.vector.memset(ones_row, 1.0)
    srep_ld = consts.tile([128, HEADS], F32, name="srep_ld")
    sx_ld = consts.tile([128, HEADS], F32, name="sx_ld")
    srep_bc = consts.tile([128, NPAIR], F32, name="srep_bc")
    sx_bc = consts.tile([128, NPAIR], F32, name="sx_bc")
    for st_dram, st_ld, st_bc in ((srep, srep_ld, srep_bc), (sx, sx_ld, sx_bc)):
        bcast = bass.AP(
            tensor=st_dram.tensor, offset=st_dram.offset,
            ap=[[0, 128], [st_dram.ap[0][0], HEADS]],
        )
        nc.sync.dma_start(st_ld, bcast)
        st_ldv = st_ld.rearrange("p (c two) -> p c two", two=2)
        for half in range(2):
            rows = slice(64 * half, 64 * half + 64)
            nc.gpsimd.tensor_copy(st_bc[rows, :], st_ldv[rows, :, half])

    def _build_repT():
        # repT = Wp^T xpool / 64; repT_bd[:, p, :] is [[sA, 0], [0, sB]]
        # (exact zeros kill the cross-head terms).  The scores then come
        # straight from x: S_p = repT_bd_p^T (Wp^T x) = (Wp repT_bd_p)^T x,
        # so Ws = Wp @ repT_bd (a [256, 128] tile per pair) replaces the
        # whole 4096-wide transposed projection -- no projT tensor at all.
        nc.vector.tensor_scalar_mul(xpool_r, xpool_raw, 1.0 / 64.0)
        nc.vector.memset(repT_bd, 0.0)
        for g in range(4):
            rt_ps = psum.tile([128, NB], F32, name="rt_ps", tag="sm", bufs=2)
            for o in range(2):
                nc.tensor.matmul(
                    rt_ps, wp_r[:, o, ts(g, 128)], xpool_r[:, o, :],
                    start=(o == 0), stop=(o == 1),
                )
            nc.vector.tensor_copy(repT[:, g, :], rt_ps)
            for h in range(2):
                rows = slice(64 * h, 64 * h + 64)
                nc.vector.tensor_scalar_mul(
                    repT_bd[rows, g, ds(64 * h, 64)], repT[rows, g, :], SCALE
                )
        nc.vector.tensor_copy(repT_bd_r, repT_bd)
        for p in range(NPAIR):
            for o in range(2):
                ws_ps = psum.tile([128, 128], F32, name="ws_ps", tag="sm",
                                  bufs=2)
                nc.tensor.matmul(
                    ws_ps, wpT_sb[:, p, ds(128 * o, 128)], repT_bd_r[:, p, :],
                    start=True, stop=True,
                )
                nc.vector.tensor_copy(ws_sb[:, o, p, :], ws_ps)

    def _issue_scores(c):
        # stage-1 scores + exp for token chunk c, all pairs, straight from
        # the staged x.  P stays unnormalized; 1/Z is folded into rep_delta
        # (via rz_bc) and into V (via rz*step_x).  |s| <~ 2: no max-subtract.
        for p in range(NPAIR):
            s_ps = psum.tile([128, 512], F32, name="s_ps", tag="smm",
                             bufs=2)
            for o in range(2):
                nc.tensor.matmul(
                    s_ps, ws_sb[:, o, p, :], x_sb[:, o, ts(c, 512)],
                    start=(o == 0), stop=(o == 1),
                )
            nc.scalar.activation(
                out=p_tiles[p][:, ts(c, 512)], in_=s_ps, func=ACTF.Exp,
                bias=0.0, scale=1.0, accum_out=zpart_tiles[p][:, c:c + 1],
            )

    def _issue_pt(q):
        # P^T transpose piece for chunk pair (2q, 2q+1), all pairs.  Issued
        # piecewise right behind the exp chain so the DMA-serial transpose
        # stream (~1us per piece) chases the exps instead of trailing them.
        for p in range(NPAIR):
            nc.sync.dma_start_transpose(
                pt_tiles[p][:, 8 * q:8 * q + 8, :],
                p_tiles[p][:, ds(1024 * q, 1024)],
            )

    # projection loop; repT built once chunk 3 is through the PE (the xpool
    # inputs are ready by then), scores+exp threaded through chunks 3-7.
    for j in range(NCHUNK):
        for m in range(4 * j, 4 * j + 4):
            # token-partition projection tile (rep_delta lhsT)
            pr_ps = psum.tile([128, INNER], F32, name="pr_ps", tag="mm",
                              bufs=mm_bufs)
            for o in range(2):
                nc.tensor.matmul(
                    pr_ps, x_sb[:, o, ts(m, 128)], wp_r[:, o, :],
                    start=(o == 0), stop=(o == 1),
                )
            # PSUM exports are DVE/ACT only (GPSIMD cannot access PSUM)
            nc.vector.tensor_copy(proj_bf[:, m, :], pr_ps)
        if j == 3:
            with tc.high_priority():
                _build_repT()
                _issue_scores(0)
                _issue_scores(1)
        elif 4 <= j < 7:
            with tc.high_priority():
                _issue_scores(2 * (j - 4) + 2)
                _issue_scores(2 * (j - 4) + 3)
        if j >= 4:
            _issue_pt(j - 4)

    # x staging is dead now; the out staging pool reuses its SBUF space
    xp.release()
    pTp.release()
    outp = tc.alloc_tile_pool(name="outp", bufs=1)

    # ---- softmax normalizers ---------------------------------------------
    rz_tiles = []
    for p in range(NPAIR):
        zsum = stats.tile([128, 1], F32, name="zsum", tag=f"zsum{p}")
        nc.vector.reduce_sum(zsum, zpart_tiles[p], axis=AX.X)
        rz = stats.tile([128, 1], F32, name="rz", tag=f"rz{p}")
        nc.vector.reciprocal(rz, zsum)
        rz_tiles.append(rz)


    # ---- rep_delta^T per pair (with rz broadcast built on the side) ------
    # rz_bc is a [128,128] tile whose every row equals rz^T (ones x rz^T).
    # rep2^T for pair p-1 (a DVE-only chain) is built while pair p's
    # rep_delta accumulates, so the single 'acc' PSUM bank recycles fast.
    rep2Tb_tiles = []
    rep2Tbd_tiles = []

    def _build_rep2(p, av1_ps, rz_bc):
        # rep2^T = repT + step_rep * rz * rep_delta^T   [128 (dA|dB), 64 q]
        rep2T = b3.tile([128, NB], F32, name="rep2T", tag="rep2T", bufs=2)
        for h in range(2):
            pr = slice(64 * h, 64 * h + 64)
            blk = ds(64 * h, 64)
            nc.vector.scalar_tensor_tensor(
                rep2T[pr, :], av1_ps[pr, blk], srep_bc[pr, p:p + 1],
                rz_bc[pr, blk], op0=ALU.mult, op1=ALU.mult,
            )
        nc.vector.tensor_add(rep2T, rep2T, repT[:, p, :])
        rep2T_b = b3.tile([128, NB], BF16, name="rep2T_b", tag="rep2T_b", bufs=4)
        nc.vector.tensor_copy(rep2T_b, rep2T)
        rep2Tb_tiles.append(rep2T_b)
        rep2T_bd = b3.tile([128, 128], BF16, name="rep2T_bd", tag="rep2T_bd",
                           bufs=4)
        nc.vector.memset(rep2T_bd, 0.0)
        for h in range(2):
            rows = slice(64 * h, 64 * h + 64)
            nc.vector.tensor_scalar_mul(
                rep2T_bd[rows, ds(64 * h, 64)], rep2T[rows, :], SCALE
            )
        rep2Tbd_tiles.append(rep2T_bd)

    pending = []
    for p in range(NPAIR):
        rzt_ps = psum.tile([1, 128], F32, name="rzt_ps", tag="sm", bufs=2)
        nc.tensor.transpose(rzt_ps, rz_tiles[p], ident_f)
        rzt_sb = b3.tile([1, 128], F32, name="rzt_sb", tag="rzt_sb", bufs=4)
        nc.vector.tensor_copy(rzt_sb, rzt_ps)
        rzbc_ps = psum.tile([128, 128], F32, name="rzbc_ps", tag="sm", bufs=2)
        nc.tensor.matmul(rzbc_ps, ones_row, rzt_sb, start=True, stop=True)
        rz_bc = b3.tile([128, 128], F32, name="rz_bc", tag="rz_bc", bufs=2)
        nc.vector.tensor_copy(rz_bc, rzbc_ps)
        # rep_delta^T: [128 (dA|dB), 128 (qA|qB)] accumulated over 32 t-tiles;
        # diagonal quadrants are the two heads' rep_delta^T.
        av1_ps = psum.tile([128, 128], F32, name="av1_ps", tag="acc", bufs=1)
        for m in range(NTT):
            nc.tensor.matmul(
                av1_ps, proj_bf[:, m, ds(128 * p, 128)], pt_tiles[p][:, m, :],
                start=(m == 0), stop=(m == NTT - 1),
            )
        pending.append((p, av1_ps, rz_bc))
        if len(pending) > 1:
            _build_rep2(*pending.pop(0))

    # ---- stage 2: small self-attention + V^T = (rz*sx*xd2_bd)^T @ Wo_pair
    r2_tiles = [None] * NPAIR
    p2_tiles = [None] * NPAIR
    rz2_tiles = [None] * NPAIR
    v_tiles = [None] * NPAIR

    def _stage2a(p):
        # rep2 (q' on partitions): [64 q', 128 (dA|dB)]
        r2_ps = psum.tile([64, 128], BF16, name="r2_ps", tag="sm", bufs=2)
        nc.tensor.transpose(r2_ps, rep2Tb_tiles[p], ident_bf)
        r2_sb = b3.tile([64, 128], BF16, name="r2_sb", tag="r2_sb", bufs=4)
        nc.vector.tensor_copy(r2_sb, r2_ps)
        r2_tiles[p] = r2_sb
        # S2 = (scale*rep2) @ rep2.T per head -> [128 (qA|qB), 64 q'].  Only
        # its exp row-sum (Z2) is consumed; exp(S2) itself is taken from the
        # transposed product below (exp(S2) is symmetric per head block).
        s2_ps = psum.tile([128, NB], F32, name="s2_ps", tag="sm", bufs=2)
        nc.tensor.matmul(s2_ps, rep2Tbd_tiles[p], rep2Tb_tiles[p],
                         start=True, stop=True)
        z2 = stats.tile([128, 1], F32, name="z2", tag=f"z2{p}")
        p2_sb = b3.tile([128, NB], BF16, name="p2_sb", tag="p2_sb", bufs=4)
        nc.scalar.activation(
            out=p2_sb, in_=s2_ps, func=ACTF.Exp,
            bias=0.0, scale=1.0, accum_out=z2,
        )
        # S2^T = rep2 @ (scale*rep2)^T -> [64 q', 128 (qA|qB)]; its exp IS
        # P2^T unnormalized (1/Z2 is folded into the V row scale).
        s2t_ps = psum.tile([64, 128], F32, name="s2t_ps", tag="sm", bufs=2)
        nc.tensor.matmul(s2t_ps, rep2Tb_tiles[p], rep2Tbd_tiles[p],
                         start=True, stop=True)
        p2t_sb = b3.tile([64, 128], BF16, name="p2t_sb", tag="p2t_sb", bufs=4)
        nc.scalar.activation(
            out=p2t_sb, in_=s2t_ps, func=ACTF.Exp, bias=0.0, scale=1.0,
        )
        p2_tiles[p] = p2t_sb
        rz2 = stats.tile([128, 1], F32, name="rz2", tag=f"rz2{p}")
        nc.vector.reciprocal(rz2, z2)
        rz2_tiles[p] = rz2

    def _stage2b(p):
        # xd2^T directly: [128 (dA|dB), 128 (qA|qB)] = rep2^T @ P2unnorm^T;
        # diag quadrants real, cross quadrants garbage (zeroed below).
        xd2t_ps = psum.tile([128, 128], F32, name="xd2t_ps", tag="sm", bufs=2)
        nc.tensor.matmul(xd2t_ps, r2_tiles[p], p2_tiles[p],
                         start=True, stop=True)
        xd2bd = b3.tile([128, 128], BF16, name="xd2bd", tag="xd2bd", bufs=4)
        nc.vector.memset(xd2bd, 0.0)
        for h in range(2):
            rows = slice(64 * h, 64 * h + 64)
            nc.vector.tensor_copy(
                xd2bd[rows, ds(64 * h, 64)], xd2t_ps[rows, ds(64 * h, 64)]
            )
        # V_pair^T[q, c] = sum_d xd2_bd^T[d, q] Wo[128p+d, c], then scale
        # rows (queries) by rz * step_x * rz2 (stage-1 and stage-2 softmax
        # normalizers both fold in here).
        v_ps = psum.tile([128, C], F32, name="v_ps", tag="sm", bufs=2)
        nc.tensor.matmul(v_ps, xd2bd, wo_bf[:, p, :], start=True, stop=True)
        rzsx = stats.tile([128, 1], F32, name="rzsx", tag=f"rzsx{p}")
        nc.vector.tensor_mul(rzsx, rz_tiles[p], sx_bc[:, p:p + 1])
        nc.vector.tensor_mul(rzsx, rzsx, rz2_tiles[p])
        v_sb = vp.tile([128, C], BF16, name=f"v{p}", tag=f"v{p}")
        nc.vector.tensor_scalar_mul(v_sb, v_ps, rzsx)
        v_tiles[p] = v_sb

    # pairs 0-2 run to completion first; pair 3's rep2 build (which waits on
    # the last rep_delta) is issued after, so it cannot head-of-line-block
    # the DVE queue for the earlier pairs.
    for p in range(NPAIR - 1):
        _stage2a(p)
    for p in range(NPAIR - 1):
        _stage2b(p)
    _build_rep2(*pending.pop(0))
    _stage2a(NPAIR - 1)
    _stage2b(NPAIR - 1)

    # ---- fused back-projection + output projection -----------------------
    # out[c, t] = sum_p (V_pair^T)^T @ P_pair + b_out; K accumulates both
    # heads of the pair (query index carries head identity on both sides).
    # Split into pair-groups {0,1} and {2,3} so the PE starts this phase as
    # soon as V_1 lands instead of waiting for the full stage-2 tail (V_3).
    for j in range(NCHUNK):
        for ct in range(2):
            op_ps = psum.tile([128, 512], F32, name="op_ps", tag="mm",
                              bufs=mm_bufs)
            for p in range(NPAIR):
                nc.tensor.matmul(
                    op_ps, v_tiles[p][:, ts(ct, 128)], p_tiles[p][:, ts(j, 512)],
                    start=(p == 0), stop=(p == NPAIR - 1),
                )
            out_sb = outp.tile([128, 512], F32, name="out_sb", tag="out_sb",
                               bufs=4)
            if (2 * j + ct) % 2 == 0:
                nc.vector.tensor_tensor(
                    out_sb, op_ps, bo_sb[:, ct:ct + 1].to_broadcast((128, 512)),
                    ALU.add,
                )
            else:
                nc.scalar.activation(
                    out=out_sb, in_=op_ps, func=ACTF.Identity,
                    bias=bo_sb[:, ct:ct + 1], scale=1.0,
                )
            nc.sync.dma_start(out_r[:, ct, ts(j, 512)], out_sb)

    psum.release()
    outp.release()
    pnp.release()
    ptp.release()
    b3.release()
    pp.release()
    vp.release()
    stats.release()
    consts.release()


_CACHE = {}


class _Runner:
    """Builds the Bass module once and keeps a single jitted shard_map
    executable alive, so repeat kernel() calls skip retracing/relowering."""

    def __init__(self):
        import jax
        from jax.sharding import Mesh, PartitionSpec
        from jax.experimental.shard_map import shard_map
        from concourse import bass2jax

        self.jax = jax
        nc = build_module()
        self.nc = nc
        bass2jax.install_neuronx_cc_hook()

        partition_name = (
            nc.partition_id_tensor.name if nc.partition_id_tensor else None
        )
        in_names, out_names, out_avals = [], [], []
        for alloc in nc.m.functions[0].allocations:
            if not isinstance(alloc, mybir.MemoryLocationSet):
                continue
            name = alloc.memorylocations[0].name
            if alloc.kind == "ExternalInput":
                if name != partition_name:
                    in_names.append(name)
            elif alloc.kind == "ExternalOutput":
                out_names.append(name)
                out_avals.append(
                    jax.core.ShapedArray(
                        tuple(alloc.tensor_shape), mybir.dt.np(alloc.dtype)
                    )
                )
        n_params = len(in_names)
        n_outs = len(out_avals)
        all_names = list(in_names) + list(out_names)
        if partition_name is not None:
            all_names.append(partition_name)
        self.in_names = in_names
        self.out_names = out_names
        self.out_avals = out_avals

        def _body(*args):
            operands = list(args)
            if partition_name is not None:
                operands.append(bass2jax.partition_id_tensor())
            outs = bass2jax._bass_exec_p.bind(
                *operands,
                out_avals=tuple(out_avals),
                in_names=tuple(all_names),
                out_names=tuple(out_names),
                lowering_input_output_aliases=(),
                sim_require_finite=True,
                sim_require_nnan=True,
                nc=nc,
            )
            return tuple(outs)

        self.body = _body
        devices = jax.devices()[:B]
        mesh = Mesh(np.asarray(devices), ("core",))
        donate = tuple(range(n_params, n_params + n_outs))
        self.sharded = jax.jit(
            shard_map(
                _body, mesh=mesh,
                in_specs=(PartitionSpec("core"),) * (n_params + n_outs),
                out_specs=(PartitionSpec("core"),) * n_outs,
                check_rep=False,
            ),
            donate_argnums=donate,
            keep_unused=True,
        )

    def run(self, in_maps):
        concat_in = [
            np.concatenate([m[name] for m in in_maps], axis=0)
            for name in self.in_names
        ]
        zeros = [
            np.zeros((B * a.shape[0], *a.shape[1:]), a.dtype) for a in self.out_avals
        ]
        out_arrs = self.sharded(*concat_in, *zeros)
        return [
            {
                name: np.asarray(out_arrs[i]).reshape(B, *self.out_avals[i].shape)[c]
                for i, name in enumerate(self.out_names)
            }
            for c in range(B)
        ]

    def bench(self, in_maps, reps=8, inner=72, base=8):
        """Time device-resident executions (no donation, operands staged once).

        Times jitted chains of `base` and `inner` back-to-back kernel
        executions; returns (per_exec_seconds, base_chain_seconds, results)
        with per_exec = (t_inner - t_base) / (inner - base), which amortizes
        away the per-dispatch round-trip of this axon-tunneled environment.
        """
        import time
        from jax.sharding import Mesh, PartitionSpec, NamedSharding
        from jax.experimental.shard_map import shard_map

        jax = self.jax
        devices = jax.devices()[:B]
        mesh = Mesh(np.asarray(devices), ("core",))
        sharding = NamedSharding(mesh, PartitionSpec("core"))
        n_ops = len(self.in_names) + len(self.out_avals)

        def chain(n):
            def f(*args):
                outs = []
                for _ in range(n):
                    outs.extend(self.body(*args))
                return tuple(outs)
            return f

        concat_in = [
            np.concatenate([m[name] for m in in_maps], axis=0)
            for name in self.in_names
        ]
        zeros = [
            np.zeros((B * a.shape[0], *a.shape[1:]), a.dtype) for a in self.out_avals
        ]
        staged = [jax.device_put(a, sharding) for a in concat_in + zeros]

        # The device is occasionally in a degraded mode where chained
        # executions serialize (~10x): retry the whole measurement with
        # freshly traced executables and keep the best estimate.
        per_exec = float("inf")
        tbase_best = float("inf")
        out1 = None
        for attempt in range(4):
            times = {}
            for n in (base, inner):
                jfn = jax.jit(
                    shard_map(
                        chain(n), mesh=mesh,
                        in_specs=(PartitionSpec("core"),) * n_ops,
                        out_specs=(PartitionSpec("core"),) * (n * len(self.out_avals)),
                        check_rep=False,
                    ),
                    keep_unused=True,
                )
                out = jfn(*staged)
                jax.block_until_ready(out)
                best = float("inf")
                for _ in range(reps):
                    t0 = time.perf_counter()
                    out = jfn(*staged)
                    jax.block_until_ready(out)
                    best = min(best, time.perf_counter() - t0)
                times[n] = best
                if n == base and out1 is None:
                    out1 = out
            est = (times[inner] - times[base]) / (inner - base)
            if est <= 0:
                est = times[inner] / inner  # noise floor: report upper bound
            per_exec = min(per_exec, est)
            tbase_best = min(tbase_best, times[base])
            if per_exec < 1.2e-4:
                break

        results = [
            {
                name: np.asarray(out1[i]).reshape(B, *self.out_avals[i].shape)[c]
                for i, name in enumerate(self.out_names)
            }
            for c in range(B)
        ]
        return per_exec, tbase_best, results


def _get_runner():
    key = CFG["p_mode"]
    if key not in _CACHE:
        _CACHE[key] = _Runner()
    return _CACHE[key]


def _make_in_maps(x, W_proj, step_rep, step_x, W_out, b_out):
    x = np.ascontiguousarray(np.asarray(x, dtype=np.float32))
    shared = {
        "w_proj": np.ascontiguousarray(np.asarray(W_proj, dtype=np.float32)),
        "w_out": np.ascontiguousarray(np.asarray(W_out, dtype=np.float32)),
        "b_out": np.ascontiguousarray(np.asarray(b_out, dtype=np.float32)),
        "s_rep": np.ascontiguousarray(
            np.asarray(step_rep, dtype=np.float32).reshape(HEADS)
        ),
        "s_x": np.ascontiguousarray(
            np.asarray(step_x, dtype=np.float32).reshape(HEADS)
        ),
    }
    return [
        {"x": np.ascontiguousarray(x[b].reshape(C, T)), **shared} for b in range(B)
    ]


def kernel(x, W_proj, step_rep, step_x, W_out, b_out):
    runner = _get_runner()
    results = runner.run(_make_in_maps(x, W_proj, step_rep, step_x, W_out, b_out))
    outs = [np.asarray(results[b]["out"]).reshape(C, 64, 64) for b in range(B)]
    return np.stack(outs, axis=0)

